# revision 3
# baseline (speedup 1.0000x reference)
"""GAT (single-head GATConv + Linear) on 8 Trainium2 NeuronCores.

Host-gather streaming design (v2):
  - The edge structure is static, so the HOST pre-expands x[src] into the
    exact per-window slot-grid order (dst-aligned: slot (w, r, p) holds the
    src-node feature row for the r-th incoming edge of the p-th dst node of
    window w; the last round of each window is the dst node itself for the
    self-loop).  The device then never gathers: it streams contiguous
    [128, 128] bf16 x-tiles, computes h = x @ W_aug on the PE per tile
    (W_aug = [W | W@att_src | W@att_dst], so each slot's h row carries its
    a_src and the self column carries a_dst), and runs the same per-window
    segment-softmax / message pipeline as v1 on the resulting slot tiles.
  - This removes the v1 bottleneck entirely: 1.06 ms of GPSIMD descriptor
    generation for dma_gather (8.2 ns per gathered row) becomes zero; all
    DMA is contiguous 512-byte descriptors (x-tiles are pair-interleaved so
    bf16 rows form 512B runs).
  - Nodes are dealt to cores round-robin from an exact degree sort, so all
    8 cores share one static window shape table R[w]; pad slots point at a
    poison row engineered so a_src = -1e8, which drives exp() to exactly 0.
  - NOTE the reference oracle's jax.ops.segment_max actually computes a
    segment SUM in the target jax version; we reproduce w = exp(e - sum_seg e)
    and den = sum w + 1e-16 to match bit-for-bit semantics.
"""
import os
import sys

import numpy as np

if "/opt/trn_rl_repo" not in sys.path:
    sys.path.insert(0, "/opt/trn_rl_repo")

import dataclasses

import concourse.bacc as bacc
import concourse.tile as tile
from concourse import mybir
from concourse.bass_utils import run_bass_kernel_spmd
from concourse.masks import make_identity

N = 50000
IN_C, HID, OUT_C = 128, 64, 32
E = 800000
NEG_SLOPE = 0.2
P = 128
NCORES = 8
HP2 = HID + 2                   # 66: h | a_src | a_dst

LOCAL_T = 49                    # windows (dst tiles) per core
LOCAL_ROWS = LOCAL_T * P        # 6272
N_LOCAL_REAL = N // NCORES      # 6250
POISON_ASRC = -1.0e8

f32 = mybir.dt.float32
bf16 = mybir.dt.bfloat16

LAST_RESULT = None  # BassKernelResults of the most recent kernel() call


# --------------------------------------------------------------------------
# host-side layout
# --------------------------------------------------------------------------

def _build_layout(src, dst):
    """Exact-degree-sorted node dealing + per-window slot grids.

    Returns (cores, W2) where W2[w] = column count of window w (edge rounds,
    optional poison pad round to make the count even, then the self column),
    and cores[c] holds the per-core node list and x-row index table.
    """
    deg = np.bincount(dst, minlength=N).astype(np.int64)
    order = np.argsort(-deg, kind="stable")

    # shared window shapes: R[w] = max degree among ranks [w*1024, (w+1)*1024)
    # (the 8 cores' windows draw from the same global rank stripe)
    R = np.zeros(LOCAL_T, np.int64)
    degs_sorted = deg[order]
    for w in range(LOCAL_T):
        lo = w * P * NCORES
        hi = min((w + 1) * P * NCORES, N)
        R[w] = degs_sorted[lo:hi].max() if lo < N else 0
    W1 = R + 1                        # + self column
    W2 = W1 + (W1 % 2)                # pad to even for 512B pair interleave
    tile_base = np.concatenate([[0], np.cumsum(W2)])
    T_TOTAL = int(tile_base[-1])

    # per-edge round index = rank within its dst group
    so = np.argsort(dst, kind="stable")
    dst_s = dst[so]
    grp_start = np.r_[0, np.flatnonzero(np.diff(dst_s)) + 1]
    grp_sizes = np.r_[np.diff(grp_start), dst_s.size - grp_start[-1]]
    r_s = np.arange(dst_s.size) - np.repeat(grp_start, grp_sizes)
    rnd = np.empty(dst_s.size, np.int64)
    rnd[so] = r_s

    cores = []
    for c in range(NCORES):
        local_nodes = order[c::NCORES]            # 6250 in degree order
        li = np.full(N, -1, np.int64)
        li[local_nodes] = np.arange(N_LOCAL_REAL)

        emask = li[dst] >= 0
        es, ed, er = src[emask], dst[emask], rnd[emask]
        lr = li[ed]
        w_e = lr // P
        p_e = lr % P

        # x-row table: rows[q] = node id feeding slot q, or -1 for poison
        rows = np.full(T_TOTAL * P, -1, np.int64)
        q = (tile_base[w_e] + er) * P + p_e
        rows[q] = es
        # self column: last tile of each window
        wl = np.arange(N_LOCAL_REAL) // P
        pl = np.arange(N_LOCAL_REAL) % P
        rows[(tile_base[wl + 1] - 1) * P + pl] = local_nodes
        cores.append(dict(local_nodes=local_nodes, rows=rows))

    return cores, W2, tile_base, T_TOTAL


# --------------------------------------------------------------------------
# device program
# --------------------------------------------------------------------------

def _bcast(ap, shape):
    new = [ap.ap[0]] + [[0, s] for s in shape]
    return dataclasses.replace(ap, ap=new)


def _build_nc(W2, T_TOTAL):
    nc = bacc.Bacc(None, target_bir_lowering=False, num_devices=NCORES)

    # x tiles, pair-interleaved: [T/2, 128, 256] where [g, c, 0:128] is the
    # transposed tile 2g and [g, c, 128:256] is tile 2g+1
    xt_in = nc.dram_tensor("xt_in", [T_TOTAL // 2, P, 2 * P], bf16,
                           kind="ExternalInput")
    w_in = nc.dram_tensor("w_in", [IN_C, HP2], bf16, kind="ExternalInput")
    wlin_in = nc.dram_tensor("wlin_in", [P, OUT_C], f32, kind="ExternalInput")
    blin_in = nc.dram_tensor("blin_in", [P, OUT_C], f32, kind="ExternalInput")
    bconv_in = nc.dram_tensor("bconv_in", [P, HID], f32, kind="ExternalInput")
    y_out = nc.dram_tensor("y_out", [LOCAL_ROWS, OUT_C], f32,
                           kind="ExternalOutput")

    with tile.TileContext(nc) as tc:
        with (
            tc.tile_pool(name="const", bufs=1) as cpool,
            tc.tile_pool(name="px", bufs=3) as px,
            tc.tile_pool(name="ph", bufs=3) as ph,
            tc.tile_pool(name="psa", bufs=4, space="PSUM") as psa,
            tc.tile_pool(name="pb", bufs=3) as pb,
            tc.tile_pool(name="psb", bufs=2, space="PSUM") as psb,
        ):
            w_sb = cpool.tile([IN_C, HP2], bf16)
            nc.sync.dma_start(w_sb[:], w_in[:])
            wlin_sb = cpool.tile([P, OUT_C], f32)
            nc.sync.dma_start(wlin_sb[:], wlin_in[:])
            blin_sb = cpool.tile([P, OUT_C], f32)
            nc.sync.dma_start(blin_sb[:], blin_in[:])
            bconv_sb = cpool.tile([P, HID], f32)
            nc.sync.dma_start(bconv_sb[:], bconv_in[:])
            ident = cpool.tile([P, P], f32)
            make_identity(nc, ident[:])

            tb = 0
            for w in range(LOCAL_T):
                Wc = int(W2[w])
                G = Wc // 2
                xw = px.tile([P, G, 2 * P], bf16, tag="xw")
                nc.sync.dma_start(xw[:], xt_in[tb:tb + G, :, :].rearrange(
                    "g c t -> c g t"))

                hs = ph.tile([P, Wc, HP2], bf16, tag="hs")
                ha = ph.tile([P, Wc, 2], f32, tag="ha")
                for g2 in range(G):
                    h_ps = psa.tile([P, 2, HP2], f32, space="PSUM", tag="hps")
                    for h in range(2):
                        lhsT = xw[:, g2, h * P:h * P + P]
                        nc.tensor.matmul(h_ps[:, h, :], lhsT, w_sb[:],
                                         start=True, stop=True)
                    nc.scalar.copy(hs[:, 2 * g2:2 * g2 + 2, :], h_ps[:])
                    nc.scalar.copy(ha[:, 2 * g2:2 * g2 + 2, :],
                                   h_ps[:, :, HID:HID + 2])
                tb += G

                # ---- segment softmax over the Wc columns (dst = partition)
                asrc = dataclasses.replace(
                    ha[:, :, 0], ap=[ha[:].ap[0], [2, Wc]])
                adst = ha[:, Wc - 1, 1:2]                   # [P, 1] f32
                e_sb = pb.tile([P, Wc], f32, tag="e")
                nc.vector.tensor_tensor(out=e_sb[:], in0=asrc,
                                        in1=_bcast(adst, [Wc]),
                                        op=mybir.AluOpType.add)
                mask = pb.tile([P, Wc], f32, tag="mask")
                nc.vector.tensor_scalar(
                    mask[:], asrc, -1.0e7, -1.0,
                    op0=mybir.AluOpType.is_gt, op1=mybir.AluOpType.mult)
                t_sb = pb.tile([P, Wc], f32, tag="t")
                nc.vector.tensor_scalar_mul(t_sb[:], e_sb[:], NEG_SLOPE)
                nc.vector.tensor_tensor(out=e_sb[:], in0=e_sb[:], in1=t_sb[:],
                                        op=mybir.AluOpType.max)
                # reference's "segment_max" is a segment SUM in this jax
                # version; reproduce m = sum_seg(e) over real slots
                nc.vector.tensor_tensor(out=t_sb[:], in0=e_sb[:], in1=mask[:],
                                        op=mybir.AluOpType.mult)
                mneg = pb.tile([P, 1], f32, tag="mneg")
                nc.vector.tensor_reduce(mneg[:], t_sb[:],
                                        axis=mybir.AxisListType.X,
                                        op=mybir.AluOpType.add)
                wgt = pb.tile([P, Wc], bf16, tag="w")
                nc.scalar.activation(wgt[:], e_sb[:],
                                     mybir.ActivationFunctionType.Exp,
                                     bias=mneg[:, 0:1])
                # den from the bf16-rounded weights so rounding cancels in
                # alpha = w/den
                den = pb.tile([P, 1], f32, tag="den")
                nc.vector.tensor_reduce(den[:], wgt[:],
                                        axis=mybir.AxisListType.X,
                                        op=mybir.AluOpType.add)

                msgsT = pb.tile([P, HID, Wc], bf16, tag="msgsT")
                hv = dataclasses.replace(
                    hs[:, :, 0:HID], ap=[hs[:].ap[0], [1, HID], [HP2, Wc]])
                w_b = dataclasses.replace(
                    wgt[:], ap=[wgt[:].ap[0], [0, HID], [1, Wc]])
                nc.gpsimd.tensor_tensor(out=msgsT[:], in0=hv, in1=w_b,
                                        op=mybir.AluOpType.mult)
                num = pb.tile([P, HID], f32, tag="num")
                nc.vector.tensor_reduce(num[:], msgsT[:],
                                        axis=mybir.AxisListType.X,
                                        op=mybir.AluOpType.add)

                rec = pb.tile([P, 1], f32, tag="rec")
                nc.vector.tensor_scalar_add(rec[:], den[:], 1e-16)
                nc.vector.reciprocal(rec[:], rec[:])
                ow = pb.tile([P, HID], f32, tag="ow")
                nc.vector.tensor_tensor(out=ow[:], in0=num[:],
                                        in1=_bcast(rec[:, 0:1], [HID]),
                                        op=mybir.AluOpType.mult)
                nc.vector.tensor_tensor(out=ow[:], in0=ow[:], in1=bconv_sb[:],
                                        op=mybir.AluOpType.add)
                nc.vector.tensor_scalar_max(ow[:], ow[:], 0.0)

                owT_ps = psb.tile([HID, P], f32, space="PSUM", tag="owT")
                nc.tensor.transpose(owT_ps[:], ow[:], ident[:])
                # K=64 matmuls alternating with PE transposes crash the device;
                # pad lhsT to K=128 (wlin rows 64:128 are zero, host-padded)
                owT = pb.tile([P, P], f32, tag="owTs")
                nc.vector.tensor_copy(owT[0:HID, :], owT_ps[:])
                nc.gpsimd.memset(owT[HID:P, :], 0.0)
                y_ps = psb.tile([P, OUT_C], f32, space="PSUM", tag="y")
                nc.tensor.matmul(y_ps[:], owT[:], wlin_sb[:],
                                 start=True, stop=True)
                y_sb = pb.tile([P, OUT_C], f32, tag="ysb")
                nc.vector.tensor_tensor(out=y_sb[:], in0=y_ps[:],
                                        in1=blin_sb[:],
                                        op=mybir.AluOpType.add)
                nc.sync.dma_start(y_out[w * P:(w + 1) * P, :], y_sb[:])

    nc.compile()
    return nc


# --------------------------------------------------------------------------
# entry point
# --------------------------------------------------------------------------

def kernel(x, edge_index, W, att_src, att_dst, bias_conv, W_lin, b_lin):
    global LAST_RESULT
    import ml_dtypes
    x = np.asarray(x, np.float32)
    edge_index = np.asarray(edge_index)
    W = np.asarray(W, np.float32)
    att_src = np.asarray(att_src, np.float32)
    att_dst = np.asarray(att_dst, np.float32)
    bias_conv = np.asarray(bias_conv, np.float32)
    W_lin = np.asarray(W_lin, np.float32)
    b_lin = np.asarray(b_lin, np.float32)
    src = np.asarray(edge_index[0], np.int64)
    dst = np.asarray(edge_index[1], np.int64)

    cores, W2, tile_base, T_TOTAL = _build_layout(src, dst)

    # poison row: x_p @ W projects to a_src = POISON_ASRC so exp() underflows
    h_t = POISON_ASRC * att_src / float(att_src @ att_src)
    x_poison = np.linalg.lstsq(W.T, h_t, rcond=None)[0].astype(np.float32)
    assert (x_poison.astype(ml_dtypes.bfloat16).astype(np.float32) @ W) \
        @ att_src < -1e6

    W_aug = np.concatenate(
        [W, (W @ att_src)[:, None], (W @ att_dst)[:, None]], axis=1
    ).astype(ml_dtypes.bfloat16)
    blin_b = np.tile(b_lin[None, :], (P, 1)).astype(np.float32)
    bconv_b = np.tile(bias_conv[None, :], (P, 1)).astype(np.float32)
    wlin_p = np.vstack([W_lin, np.zeros((P - HID, OUT_C), np.float32)])

    nc = _build_nc(W2, T_TOTAL)

    x_bf = x.astype(ml_dtypes.bfloat16)
    xp_bf = x_poison.astype(ml_dtypes.bfloat16)

    in_maps = []
    for cc in cores:
        rows = cc["rows"]
        xt = np.empty((T_TOTAL * P, IN_C), ml_dtypes.bfloat16)
        real = rows >= 0
        xt[real] = x_bf[rows[real]]
        xt[~real] = xp_bf
        # per-tile transpose + pair interleave: [T/2, 128c, 2*128slots]
        xt = xt.reshape(T_TOTAL // 2, 2, P, IN_C).transpose(0, 3, 1, 2) \
               .reshape(T_TOTAL // 2, IN_C, 2 * P)
        xt = np.ascontiguousarray(xt)
        in_maps.append({
            "xt_in": xt, "w_in": W_aug, "wlin_in": wlin_p,
            "blin_in": blin_b, "bconv_in": bconv_b,
        })

    res = run_bass_kernel_spmd(nc, in_maps, core_ids=list(range(NCORES)))
    LAST_RESULT = res

    y = np.empty((N, OUT_C), np.float32)
    for c, cc in enumerate(cores):
        yc = np.asarray(res.results[c]["y_out"])
        y[cc["local_nodes"]] = yc[0:N_LOCAL_REAL]
    return y


# revision 6
# speedup vs baseline: 1.2267x; 1.2267x over previous
"""GAT (single-head GATConv + Linear) on 8 Trainium2 NeuronCores.

Host-gather streaming design (v2):
  - The edge structure is static, so the HOST pre-expands x[src] into the
    exact per-window slot-grid order (dst-aligned: slot (w, r, p) holds the
    src-node feature row for the r-th incoming edge of the p-th dst node of
    window w; the last round of each window is the dst node itself for the
    self-loop).  The device then never gathers: it streams contiguous
    [128, 128] bf16 x-tiles, computes h = x @ W_aug on the PE per tile
    (W_aug = [W | W@att_src | W@att_dst], so each slot's h row carries its
    a_src and the self column carries a_dst), and runs the same per-window
    segment-softmax / message pipeline as v1 on the resulting slot tiles.
  - This removes the v1 bottleneck entirely: 1.06 ms of GPSIMD descriptor
    generation for dma_gather (8.2 ns per gathered row) becomes zero; all
    DMA is contiguous 512-byte descriptors (x-tiles are pair-interleaved so
    bf16 rows form 512B runs).
  - Nodes are dealt to cores round-robin from an exact degree sort, so all
    8 cores share one static window shape table R[w]; pad slots point at a
    poison row engineered so a_src = -1e8, which drives exp() to exactly 0.
  - NOTE the reference oracle's jax.ops.segment_max actually computes a
    segment SUM in the target jax version; we reproduce w = exp(e - sum_seg e)
    and den = sum w + 1e-16 to match bit-for-bit semantics.
"""
import os
import sys

import numpy as np

if "/opt/trn_rl_repo" not in sys.path:
    sys.path.insert(0, "/opt/trn_rl_repo")

import dataclasses

import concourse.bacc as bacc
import concourse.tile as tile
from concourse import mybir
from concourse.bass_utils import run_bass_kernel_spmd
from concourse.masks import make_identity

N = 50000
IN_C, HID, OUT_C = 128, 64, 32
E = 800000
NEG_SLOPE = 0.2
P = 128
NCORES = 8
HP2 = HID + 2                   # 66: h | a_src | a_dst

LOCAL_T = 49                    # windows (dst tiles) per core
LOCAL_ROWS = LOCAL_T * P        # 6272
N_LOCAL_REAL = N // NCORES      # 6250
POISON_ASRC = -1.0e8

f32 = mybir.dt.float32
bf16 = mybir.dt.bfloat16

LAST_RESULT = None  # BassKernelResults of the most recent kernel() call


# --------------------------------------------------------------------------
# host-side layout
# --------------------------------------------------------------------------

def _build_layout(src, dst):
    """Exact-degree-sorted node dealing + per-window slot grids.

    Returns (cores, W2) where W2[w] = column count of window w (edge rounds,
    optional poison pad round to make the count even, then the self column),
    and cores[c] holds the per-core node list and x-row index table.
    """
    deg = np.bincount(dst, minlength=N).astype(np.int64)
    order = np.argsort(-deg, kind="stable")

    # shared window shapes: R[w] = max degree among ranks [w*1024, (w+1)*1024)
    # (the 8 cores' windows draw from the same global rank stripe)
    R = np.zeros(LOCAL_T, np.int64)
    degs_sorted = deg[order]
    for w in range(LOCAL_T):
        lo = w * P * NCORES
        hi = min((w + 1) * P * NCORES, N)
        R[w] = degs_sorted[lo:hi].max() if lo < N else 0
    W1 = R + 1                        # + self column
    W2 = W1 + (W1 % 2)                # pad to even for 512B pair interleave
    tile_base = np.concatenate([[0], np.cumsum(W2)])
    T_TOTAL = int(tile_base[-1])

    # per-edge round index = rank within its dst group
    so = np.argsort(dst, kind="stable")
    dst_s = dst[so]
    grp_start = np.r_[0, np.flatnonzero(np.diff(dst_s)) + 1]
    grp_sizes = np.r_[np.diff(grp_start), dst_s.size - grp_start[-1]]
    r_s = np.arange(dst_s.size) - np.repeat(grp_start, grp_sizes)
    rnd = np.empty(dst_s.size, np.int64)
    rnd[so] = r_s

    cores = []
    for c in range(NCORES):
        local_nodes = order[c::NCORES]            # 6250 in degree order
        li = np.full(N, -1, np.int64)
        li[local_nodes] = np.arange(N_LOCAL_REAL)

        emask = li[dst] >= 0
        es, ed, er = src[emask], dst[emask], rnd[emask]
        lr = li[ed]
        w_e = lr // P
        p_e = lr % P

        # x-row table: rows[q] = node id feeding slot q, or -1 for poison
        rows = np.full(T_TOTAL * P, -1, np.int64)
        q = (tile_base[w_e] + er) * P + p_e
        rows[q] = es
        # self column: last tile of each window
        wl = np.arange(N_LOCAL_REAL) // P
        pl = np.arange(N_LOCAL_REAL) % P
        rows[(tile_base[wl + 1] - 1) * P + pl] = local_nodes
        cores.append(dict(local_nodes=local_nodes, rows=rows))

    return cores, W2, tile_base, T_TOTAL


# --------------------------------------------------------------------------
# device program
# --------------------------------------------------------------------------

def _bcast(ap, shape):
    new = [ap.ap[0]] + [[0, s] for s in shape]
    return dataclasses.replace(ap, ap=new)


def _build_nc(W2, T_TOTAL):
    nc = bacc.Bacc(None, target_bir_lowering=False, num_devices=NCORES)

    # x tiles, pair-interleaved: [T/2, 128, 256] where [g, c, 0:128] is the
    # transposed tile 2g and [g, c, 128:256] is tile 2g+1
    xt_in = nc.dram_tensor("xt_in", [T_TOTAL // 2, P, 2 * P], bf16,
                           kind="ExternalInput")
    w_in = nc.dram_tensor("w_in", [IN_C, HP2], bf16, kind="ExternalInput")
    wlin_in = nc.dram_tensor("wlin_in", [P, OUT_C], f32, kind="ExternalInput")
    blin_in = nc.dram_tensor("blin_in", [P, OUT_C], f32, kind="ExternalInput")
    bconv_in = nc.dram_tensor("bconv_in", [P, HID], f32, kind="ExternalInput")
    y_out = nc.dram_tensor("y_out", [LOCAL_ROWS, OUT_C], f32,
                           kind="ExternalOutput")

    with tile.TileContext(nc) as tc:
        with (
            tc.tile_pool(name="const", bufs=1) as cpool,
            tc.tile_pool(name="px", bufs=4) as px,
            tc.tile_pool(name="ph", bufs=4) as ph,
            tc.tile_pool(name="psa", bufs=3, space="PSUM") as psa,
            tc.tile_pool(name="pb", bufs=4) as pb,
            tc.tile_pool(name="psb", bufs=2, space="PSUM") as psb,
        ):
            w_sb = cpool.tile([IN_C, HP2], bf16)
            nc.sync.dma_start(w_sb[:], w_in[:])
            wlin_sb = cpool.tile([P, OUT_C], f32)
            nc.sync.dma_start(wlin_sb[:], wlin_in[:])
            blin_sb = cpool.tile([P, OUT_C], f32)
            nc.sync.dma_start(blin_sb[:], blin_in[:])
            bconv_sb = cpool.tile([P, HID], f32)
            nc.sync.dma_start(bconv_sb[:], bconv_in[:])
            ident = cpool.tile([P, P], f32)
            make_identity(nc, ident[:])

            tb = 0
            for w in range(LOCAL_T):
                Wc = int(W2[w])
                G = Wc // 2
                xw = px.tile([P, G, 2 * P], bf16, tag="xw")
                nc.sync.dma_start(xw[:], xt_in[tb:tb + G, :, :].rearrange(
                    "g c t -> c g t"))

                hs = ph.tile([P, Wc, HP2], bf16, tag="hs")
                ha = ph.tile([P, Wc, 2], f32, tag="ha")
                j = 0
                while j < Wc:
                    k = min(4, Wc - j)          # 4 tiles per PSUM bank
                    h_ps = psa.tile([P, 4, P], f32, space="PSUM", tag="hps")
                    for h in range(k):
                        lhsT = xw[:, (j + h) // 2, ((j + h) % 2) * P:
                                  ((j + h) % 2) * P + P]
                        nc.tensor.matmul(h_ps[:, h, 0:HP2], lhsT, w_sb[:],
                                         start=True, stop=True)
                    nc.scalar.copy(hs[:, j:j + k, :], h_ps[:, 0:k, 0:HP2])
                    nc.scalar.copy(ha[:, j:j + k, :],
                                   h_ps[:, 0:k, HID:HID + 2])
                    j += k
                tb += G

                # ---- segment softmax over the Wc columns (dst = partition)
                asrc = dataclasses.replace(
                    ha[:, :, 0], ap=[ha[:].ap[0], [2, Wc]])
                adst = ha[:, Wc - 1, 1:2]                   # [P, 1] f32
                e_sb = pb.tile([P, Wc], f32, tag="e")
                nc.vector.tensor_tensor(out=e_sb[:], in0=asrc,
                                        in1=_bcast(adst, [Wc]),
                                        op=mybir.AluOpType.add)
                mask = pb.tile([P, Wc], f32, tag="mask")
                nc.vector.tensor_scalar(
                    mask[:], asrc, -1.0e7, -1.0,
                    op0=mybir.AluOpType.is_gt, op1=mybir.AluOpType.mult)
                t_sb = pb.tile([P, Wc], f32, tag="t")
                nc.vector.tensor_scalar_mul(t_sb[:], e_sb[:], NEG_SLOPE)
                nc.vector.tensor_tensor(out=e_sb[:], in0=e_sb[:], in1=t_sb[:],
                                        op=mybir.AluOpType.max)
                # reference's "segment_max" is a segment SUM in this jax
                # version; reproduce m = sum_seg(e) over real slots
                nc.vector.tensor_tensor(out=t_sb[:], in0=e_sb[:], in1=mask[:],
                                        op=mybir.AluOpType.mult)
                mneg = pb.tile([P, 1], f32, tag="mneg")
                nc.vector.tensor_reduce(mneg[:], t_sb[:],
                                        axis=mybir.AxisListType.X,
                                        op=mybir.AluOpType.add)
                wgt = pb.tile([P, Wc], bf16, tag="w")
                nc.scalar.activation(wgt[:], e_sb[:],
                                     mybir.ActivationFunctionType.Exp,
                                     bias=mneg[:, 0:1])
                # den from the bf16-rounded weights so rounding cancels in
                # alpha = w/den
                den = pb.tile([P, 1], f32, tag="den")
                nc.vector.tensor_reduce(den[:], wgt[:],
                                        axis=mybir.AxisListType.X,
                                        op=mybir.AluOpType.add)

                msgsT = pb.tile([P, HID, Wc], bf16, tag="msgsT")
                hv = dataclasses.replace(
                    hs[:, :, 0:HID], ap=[hs[:].ap[0], [1, HID], [HP2, Wc]])
                w_b = dataclasses.replace(
                    wgt[:], ap=[wgt[:].ap[0], [0, HID], [1, Wc]])
                nc.gpsimd.tensor_tensor(out=msgsT[:], in0=hv, in1=w_b,
                                        op=mybir.AluOpType.mult)
                num = pb.tile([P, HID], f32, tag="num")
                nc.vector.tensor_reduce(num[:], msgsT[:],
                                        axis=mybir.AxisListType.X,
                                        op=mybir.AluOpType.add)

                rec = pb.tile([P, 1], f32, tag="rec")
                nc.vector.tensor_scalar_add(rec[:], den[:], 1e-16)
                nc.vector.reciprocal(rec[:], rec[:])
                ow = pb.tile([P, HID], f32, tag="ow")
                nc.vector.tensor_tensor(out=ow[:], in0=num[:],
                                        in1=_bcast(rec[:, 0:1], [HID]),
                                        op=mybir.AluOpType.mult)
                nc.vector.tensor_tensor(out=ow[:], in0=ow[:], in1=bconv_sb[:],
                                        op=mybir.AluOpType.add)
                nc.vector.tensor_scalar_max(ow[:], ow[:], 0.0)

                owT_ps = psb.tile([HID, P], f32, space="PSUM", tag="owT")
                nc.tensor.transpose(owT_ps[:], ow[:], ident[:])
                # K=64 matmuls alternating with PE transposes crash the device;
                # pad lhsT to K=128 (wlin rows 64:128 are zero, host-padded)
                owT = pb.tile([P, P], f32, tag="owTs")
                nc.vector.tensor_copy(owT[0:HID, :], owT_ps[:])
                nc.gpsimd.memset(owT[HID:P, :], 0.0)
                y_ps = psb.tile([P, OUT_C], f32, space="PSUM", tag="y")
                nc.tensor.matmul(y_ps[:], owT[:], wlin_sb[:],
                                 start=True, stop=True)
                y_sb = pb.tile([P, OUT_C], f32, tag="ysb")
                nc.vector.tensor_tensor(out=y_sb[:], in0=y_ps[:],
                                        in1=blin_sb[:],
                                        op=mybir.AluOpType.add)
                nc.sync.dma_start(y_out[w * P:(w + 1) * P, :], y_sb[:])

    nc.compile()
    return nc


# --------------------------------------------------------------------------
# entry point
# --------------------------------------------------------------------------

def kernel(x, edge_index, W, att_src, att_dst, bias_conv, W_lin, b_lin):
    global LAST_RESULT
    import ml_dtypes
    x = np.asarray(x, np.float32)
    edge_index = np.asarray(edge_index)
    W = np.asarray(W, np.float32)
    att_src = np.asarray(att_src, np.float32)
    att_dst = np.asarray(att_dst, np.float32)
    bias_conv = np.asarray(bias_conv, np.float32)
    W_lin = np.asarray(W_lin, np.float32)
    b_lin = np.asarray(b_lin, np.float32)
    src = np.asarray(edge_index[0], np.int64)
    dst = np.asarray(edge_index[1], np.int64)

    cores, W2, tile_base, T_TOTAL = _build_layout(src, dst)

    # poison row: x_p @ W projects to a_src = POISON_ASRC so exp() underflows
    h_t = POISON_ASRC * att_src / float(att_src @ att_src)
    x_poison = np.linalg.lstsq(W.T, h_t, rcond=None)[0].astype(np.float32)
    assert (x_poison.astype(ml_dtypes.bfloat16).astype(np.float32) @ W) \
        @ att_src < -1e6

    W_aug = np.concatenate(
        [W, (W @ att_src)[:, None], (W @ att_dst)[:, None]], axis=1
    ).astype(ml_dtypes.bfloat16)
    blin_b = np.tile(b_lin[None, :], (P, 1)).astype(np.float32)
    bconv_b = np.tile(bias_conv[None, :], (P, 1)).astype(np.float32)
    wlin_p = np.vstack([W_lin, np.zeros((P - HID, OUT_C), np.float32)])

    nc = _build_nc(W2, T_TOTAL)

    x_bf = x.astype(ml_dtypes.bfloat16)
    xp_bf = x_poison.astype(ml_dtypes.bfloat16)

    in_maps = []
    for cc in cores:
        rows = cc["rows"]
        xt = np.empty((T_TOTAL * P, IN_C), ml_dtypes.bfloat16)
        real = rows >= 0
        xt[real] = x_bf[rows[real]]
        xt[~real] = xp_bf
        # per-tile transpose + pair interleave: [T/2, 128c, 2*128slots]
        xt = xt.reshape(T_TOTAL // 2, 2, P, IN_C).transpose(0, 3, 1, 2) \
               .reshape(T_TOTAL // 2, IN_C, 2 * P)
        xt = np.ascontiguousarray(xt)
        in_maps.append({
            "xt_in": xt, "w_in": W_aug, "wlin_in": wlin_p,
            "blin_in": blin_b, "bconv_in": bconv_b,
        })

    res = run_bass_kernel_spmd(nc, in_maps, core_ids=list(range(NCORES)))
    LAST_RESULT = res

    y = np.empty((N, OUT_C), np.float32)
    for c, cc in enumerate(cores):
        yc = np.asarray(res.results[c]["y_out"])
        y[cc["local_nodes"]] = yc[0:N_LOCAL_REAL]
    return y
